# revision 3
# baseline (speedup 1.0000x reference)
"""Trainium2 Bass kernel for nn_ExpertsChooseMaskedExpand (MoE routing).

Reference computes (per batch b):
    xd[e,c,j] = sum_t mask[t,e,c] * x[t,e,j]          (dispatch)
    y[e,c,o]  = sum_j xd[e,c,j] * w[e,o,j] + bias[o]  (expert GEMM)
    out[t,o]  = sum_{e,c} comb[t,e,c] * y[e,c,o]      (combine)

We use associativity to contract comb with xd first:
    z[t,e,j] = sum_c comb[t,e,c] * xd[e,c,j]
    out[t,o] = sum_{e,j} z[t,e,j] * w[e,o,j] + bias[o] * S[t],
    S[t] = sum_{e,c} comb[t,e,c]
which cuts FLOPs ~3.4x and never materializes y (B,E,C,O).

Sharding: 8 cores; core k handles batch b=k//2, token half h=k%2 (2048
tokens). Each core computes its batch's full xd locally (dispatch work
duplicated across the pair) so no cross-core communication is needed.
All matmuls run in bf16 with fp32 PSUM accumulation; inputs are cast and
re-laid-out on the host so every DMA is wide and contiguous.
"""

import numpy as np
import ml_dtypes

BF16 = ml_dtypes.bfloat16

B, T, E, C = 4, 4096, 8, 512
I = 128            # per-expert input features
O = 4096           # out_features
NCORES = 8
TLOC = B * T // NCORES      # 2048 tokens per core
NTT = T // 128              # 32 token tiles per batch (dispatch)
NQ = 4                      # dispatch chunk groups (8 token-tiles each)
NCT = C // 128              # 4 c-tiles
NTC = TLOC // 512           # 4 t-chunks per core (z stage)
NOT = O // 512              # 8 o-tiles
NTTL = TLOC // 128          # 16 local token tiles (final stage)

_CACHE = {}


def _build():
    import concourse.bass as bass
    import concourse.tile as tile
    import concourse.bacc as bacc
    import concourse.mybir as mybir

    f32 = mybir.dt.float32
    bf16 = mybir.dt.bfloat16
    ts = bass.ts

    nc = bacc.Bacc(None, target_bir_lowering=False, debug=False)

    xh = nc.dram_tensor("xh", [E, 128, NTT, I], bf16, kind="ExternalInput")
    mh = nc.dram_tensor("mh", [E, 128, NTT, C], bf16, kind="ExternalInput")
    cbt = nc.dram_tensor("cbt", [E, C, TLOC], bf16, kind="ExternalInput")
    wf = nc.dram_tensor("wf", [128, E, O], bf16, kind="ExternalInput")
    bias2 = nc.dram_tensor("bias2", [1, O], bf16, kind="ExternalInput")
    s2 = nc.dram_tensor("s2", [1, TLOC], bf16, kind="ExternalInput")
    out_d = nc.dram_tensor("out", [TLOC, O], f32, kind="ExternalOutput")

    with tile.TileContext(nc) as tc:
        with (
            tc.tile_pool(name="stream", bufs=1) as stream,
            tc.tile_pool(name="persist", bufs=1) as persist,
            tc.tile_pool(name="psum", bufs=1, space="PSUM") as psum,
        ):
            # Resident tensors
            wf_sb = persist.tile([128, E, O], bf16, tag="wf")
            nc.sync.dma_start(wf_sb[:], wf[:])
            bias_sb = persist.tile([1, O], bf16, tag="bias")
            nc.sync.dma_start(bias_sb[:], bias2[:])
            s_sb = persist.tile([1, TLOC], bf16, tag="s")
            nc.sync.dma_start(s_sb[:], s2[:])

            zt = {}  # (e, tc) -> z^T tile [128j, 512t] bf16
            for e in range(E):
                # ---- Stage A: dispatch xd[e] = mask^T @ x (contract t) ----
                # One PSUM bank per ct: start=True zeroes a whole 2KB zero
                # region, so accumulation groups must not share a bank.
                ps_a = [psum.tile([128, 128], f32, tag="psA", bufs=4,
                                  name=f"psA{e}_{ct}") for ct in range(NCT)]
                for q in range(NQ):
                    mh_t = stream.tile([128, 8, C], bf16, tag="mh", bufs=3,
                                       name=f"mh{e}_{q}")
                    nc.sync.dma_start(mh_t[:], mh[e, :, q * 8:(q + 1) * 8, :])
                    xh_t = stream.tile([128, 8, I], bf16, tag="xh", bufs=3,
                                       name=f"xh{e}_{q}")
                    nc.sync.dma_start(xh_t[:], xh[e, :, q * 8:(q + 1) * 8, :])
                    for i in range(8):
                        tt = q * 8 + i
                        for ct in range(NCT):
                            nc.tensor.matmul(
                                ps_a[ct][:],
                                mh_t[:, i, ts(ct, 128)],
                                xh_t[:, i, :],
                                start=(tt == 0),
                                stop=(tt == NTT - 1),
                            )
                xd_sb = stream.tile([128, C], bf16, tag="xd", bufs=2,
                                    name=f"xd{e}")
                for ct in range(NCT):
                    nc.vector.tensor_copy(xd_sb[:, ts(ct, 128)], ps_a[ct][:])

                # ---- Stage B: z^T[e] = xd[e]^T-contract-c comb^T[e] ----
                cb_t = stream.tile([128, NCT, TLOC], bf16, tag="cb", bufs=2,
                                   name=f"cb{e}")
                for ct in range(NCT):
                    nc.sync.dma_start(cb_t[:, ct, :],
                                      cbt[e, ts(ct, 128), :])
                for tch in range(NTC):
                    ps_b = psum.tile([128, 512], f32, tag="psB", bufs=2,
                                     name=f"psB{e}_{tch}")
                    for ct in range(NCT):
                        nc.tensor.matmul(
                            ps_b[:],
                            xd_sb[:, ts(ct, 128)],
                            cb_t[:, ct, ts(tch, 512)],
                            start=(ct == 0),
                            stop=(ct == NCT - 1),
                        )
                    z_sb = persist.tile([128, 512], bf16, tag=f"zt{e}_{tch}",
                                        name=f"zt{e}_{tch}")
                    nc.vector.tensor_copy(z_sb[:], ps_b[:])
                    zt[(e, tch)] = z_sb

            # ---- Stage C: out[t,o] = sum_{e,j} z^T[j,t] w[e,o,j] + S bias ----
            for tt in range(NTTL):
                tch, m = tt // 4, tt % 4
                out_sb = stream.tile([128, O], f32, tag="out", bufs=2,
                                     name=f"out{tt}")
                for ot in range(NOT):
                    ps_c = psum.tile([128, 512], f32, tag="psC", bufs=2,
                                     name=f"psC{tt}_{ot}")
                    nc.tensor.matmul(
                        ps_c[:],
                        s_sb[0:1, ts(tt, 128)],
                        bias_sb[0:1, ts(ot, 512)],
                        start=True,
                        stop=False,
                    )
                    for e in range(E):
                        nc.tensor.matmul(
                            ps_c[:],
                            zt[(e, tch)][:, ts(m, 128)],
                            wf_sb[:, e, ts(ot, 512)],
                            start=False,
                            stop=(e == E - 1),
                        )
                    nc.vector.tensor_copy(out_sb[:, ts(ot, 512)], ps_c[:])
                nc.scalar.dma_start(out_d[ts(tt, 128), :], out_sb[:])

    nc.compile()
    return nc


def _prep_inputs(x, weight, bias, combine_array, dispatch_mask):
    """Host-side cast to bf16 + re-layout for contiguous device DMA."""
    x = np.asarray(x, np.float32)
    weight = np.asarray(weight, np.float32)
    bias = np.asarray(bias, np.float32)
    comb = np.asarray(combine_array, np.float32)
    mask = np.asarray(dispatch_mask, np.float32)

    # xh[b]: (E, 128, NTT, I); xh[b][e, p, tt, j] = x[b, tt*128+p, e, j]
    xh = np.ascontiguousarray(
        x.reshape(B, NTT, 128, E, I).transpose(0, 3, 2, 1, 4)).astype(BF16)
    # mh[b]: (E, 128, NTT, C)
    mh = np.ascontiguousarray(
        mask.reshape(B, NTT, 128, E, C).transpose(0, 3, 2, 1, 4)).astype(BF16)
    # cbt[b][h]: (E, C, TLOC); cbt[...][e, c, t] = comb[b, h*TLOC+t, e, c]
    cbt = np.ascontiguousarray(
        comb.reshape(B, 2, TLOC, E, C).transpose(0, 1, 3, 4, 2)).astype(BF16)
    # wf: (128, E, O); wf[j, e, o] = weight.reshape(E, O, I)[e, o, j]
    wf = np.ascontiguousarray(
        weight.reshape(E, O, I).transpose(2, 0, 1)).astype(BF16)
    bias2 = bias.reshape(1, O).astype(BF16)
    # S[b, t] = sum_{e,c} comb[b, t, e, c]
    s = comb.sum(axis=(2, 3)).reshape(B, 2, 1, TLOC).astype(BF16)

    in_maps = []
    for k in range(NCORES):
        b, h = k // 2, k % 2
        in_maps.append({
            "xh": xh[b], "mh": mh[b], "cbt": cbt[b, h],
            "wf": wf, "bias2": bias2, "s2": s[b, h],
        })
    return in_maps


def kernel(x, weight, bias, combine_array, dispatch_mask):
    from concourse import bass_utils

    if "nc" not in _CACHE:
        _CACHE["nc"] = _build()
    nc = _CACHE["nc"]

    in_maps = _prep_inputs(x, weight, bias, combine_array, dispatch_mask)
    res = bass_utils.run_bass_kernel_spmd(
        nc, in_maps, core_ids=list(range(NCORES)))
    out = np.stack([res.results[k]["out"] for k in range(NCORES)])
    return out.reshape(B, T, O).astype(np.float32)
